# revision 7
# baseline (speedup 1.0000x reference)
"""Trainium2 Bass kernel for nn_MeanAggregator (segment mean + time features).

Computation (see reference):
  out[e, p, 0:256]   = mean of 10 gathered ent_embeds rows of segment 5e+p (p<5)
  out[e, p, 256:288] = cos(t * t_w + t_b), t = time_vals[5e+p]             (p<5)
  out[e, p, 0:256]   = 0,  out[e, p, 256:288] = cos(1e6*t_w + t_b)         (p>=5)

Sharding: data-parallel by segment range; core c owns segments
[12500c, 12500(c+1)) == examples [2500c, 2500(c+1)).

Hardware reality (measured): every data-dependent DMA path (indirect DMA,
dma_gather) is emission-bound on the Q7 SWDGE at ~8.6 ns/descriptor, one
row per descriptor, engine-serial.  125k gathered rows/core -> ~1.07 ms of
GpSimd time is the floor; the job of the rest of the kernel is to stay out
of the way.  Design:
  - Table staged as bf16 (+1 trailing zero row for absent segments).
  - Host reorders the node stream block-major: block (B, j) = seq-pos j of
    examples [128B, 128B+128); its 1280 node rows are slot-major, so row r
    of the block belongs to output slot r//10.  That mapping is STATIC:
    ten precomputed one-hot matrices M_t[p, s] = ((128t+p)//10 == s) turn
    the per-block segment sum into 10 PSUM-accumulated TensorE matmuls.
    No per-tile index compute on DVE at all.
  - Gathers are [128,1]-offset indirect DMAs (the only HW-honored form),
    one per block tile: 10 per block, partial tail tiles for the last
    example block, exactly 125000 descriptors per core.
  - ScalarE evicts PSUM * 0.1 into a [128, 10, 288] per-example-block tile
    (plus host-LUT time features and pad halves); one 1.44 MB DMA per
    example block writes the output.
"""

import math
import os
import sys

import numpy as np

sys.path.insert(0, "/opt/trn_rl_repo")

from contextlib import ExitStack

import ml_dtypes

import concourse.bass as bass
import concourse.tile as tile
from concourse import bacc, mybir
from concourse._compat import with_exitstack
from concourse.bass_utils import run_bass_kernel_spmd

# Problem constants (hardcoded; kernel.py must be self-contained).
N_CORES = 8
NUM_ENTITIES = 200000
H = 256
T = 32
SEQ_LEN = 10
N_EXAMPLES = 20000
SEGS_PER_EX = 5
NODES_PER_SEG = 10
N_SEG = N_EXAMPLES * SEGS_PER_EX
N_NODES = N_SEG * NODES_PER_SEG
EX_PER_CORE = N_EXAMPLES // N_CORES      # 2500
NSEG_CORE = N_SEG // N_CORES             # 12500
P = 128
NB_EX = (EX_PER_CORE + P - 1) // P       # 20 example blocks per core
ZROW = NUM_ENTITIES                       # index of the staged all-zero row
PAD_TIME = 1000000.0

_CACHE = {}


class _Plan:
    def __init__(self, block_keys, block_Bj, nb, npar_of_B):
        self.block_keys = block_keys
        self.block_Bj = block_Bj
        self.nb = nb
        self.npar_of_B = npar_of_B


def _host_prep(t_w, t_b, flat_s, node_seg_ids, seg_example, seg_pos, time_vals):
    """Shared static plan + per-core device input arrays."""
    e = seg_example.astype(np.int64)
    j = seg_pos.astype(np.int64)
    segs = np.arange(N_SEG, dtype=np.int64)
    core_of_seg = segs // NSEG_CORE
    e_loc = e - core_of_seg * EX_PER_CORE
    assert e_loc.min() >= 0 and e_loc.max() < EX_PER_CORE, (
        "segment's example outside its core's range; resharding needed"
    )
    B = e_loc // P
    slot = e_loc % P
    segkey = B * SEQ_LEN + j
    uniq = ((core_of_seg * NB_EX + B) * SEQ_LEN + j) * P + slot
    assert np.unique(uniq).size == N_SEG, "duplicate (example, pos) targets"

    block_keys = np.unique(segkey)
    nb = int(block_keys.size)
    bi_of_key = np.full(NB_EX * SEQ_LEN, -1, np.int64)
    bi_of_key[block_keys] = np.arange(nb)
    seg_bi = bi_of_key[segkey]

    # node rows per segment (node_seg_ids is sorted)
    nseg = node_seg_ids.astype(np.int64)
    starts = np.searchsorted(nseg, segs)
    ends = np.searchsorted(nseg, segs, side="right")
    assert ((ends - starts) == NODES_PER_SEG).all(), (
        "kernel assumes exactly 10 nodes per segment"
    )

    # per-core block-major node stream: idx_blk[core][bi, slot, node]
    fs = flat_s.astype(np.int32)
    idx_blk = np.full((N_CORES, nb, P, NODES_PER_SEG), ZROW, np.int32)
    node_rows = starts[:, None] + np.arange(NODES_PER_SEG)[None, :]  # [N_SEG,10]
    idx_blk[core_of_seg, seg_bi, slot] = fs[node_rows]

    # gather-call layout: idx_res[p, bi*10 + t] = stream row 128t+p of block
    # (stream = slot-major flatten of [P, 10])
    idx_hosts = []
    for c in range(N_CORES):
        stream = idx_blk[c].reshape(nb, P * NODES_PER_SEG)       # [nb, 1280]
        tiles = stream.reshape(nb, SEQ_LEN, P)                    # [nb, 10t, 128p]
        idx_hosts.append(
            np.ascontiguousarray(
                np.transpose(tiles, (2, 0, 1)).reshape(P, nb * SEQ_LEN)
            )
        )

    # static one-hot M_t[p, s] = ((128t+p)//10 == s)
    m_host = np.zeros((P, SEQ_LEN, P), np.float32)
    for t in range(SEQ_LEN):
        r = 128 * t + np.arange(P)
        s = r // NODES_PER_SEG
        ok = s < P
        m_host[np.arange(P)[ok], t, s[ok]] = 1.0
    m_host = m_host.astype(ml_dtypes.bfloat16)

    # time features per (core, block): [P, nb, T] f32, pad_vec default
    t_w32 = t_w.astype(np.float32)
    t_b32 = t_b.astype(np.float32)
    pad_vec = np.cos(np.float32(PAD_TIME) * t_w32 + t_b32).astype(np.float32)
    tf_seg = np.cos(
        time_vals.astype(np.float32)[:, None] * t_w32 + t_b32
    ).astype(np.float32)
    tf_all = np.tile(pad_vec, (N_CORES, P, nb, 1)).astype(np.float32)
    tf_all[core_of_seg, slot, seg_bi] = tf_seg
    tf_hosts = [np.ascontiguousarray(tf_all[c]) for c in range(N_CORES)]

    pad_host = np.zeros((P, H + T), np.float32)
    pad_host[:, H:] = pad_vec

    block_Bj = [(int(k) // SEQ_LEN, int(k) % SEQ_LEN) for k in block_keys]
    npar_of_B = [min(P, EX_PER_CORE - Bx * P) for Bx in range(NB_EX)]
    plan = _Plan(block_keys, block_Bj, nb, npar_of_B)
    return plan, idx_hosts, m_host, tf_hosts, pad_host


@with_exitstack
def _emit(ctx: ExitStack, tc, plan, table, idxr, mr, tfr, padr, out):
    nc = tc.nc
    f32 = mybir.dt.float32
    bf16 = mybir.dt.bfloat16
    nb = plan.nb

    const_pool = ctx.enter_context(tc.tile_pool(name="const", bufs=1))
    g_pool = ctx.enter_context(tc.tile_pool(name="g", bufs=6))
    ob_pool = ctx.enter_context(tc.tile_pool(name="ob", bufs=3))
    ps_pool = ctx.enter_context(tc.tile_pool(name="ps", bufs=4, space="PSUM"))

    idx_res = const_pool.tile([P, nb * SEQ_LEN], mybir.dt.int32)
    nc.sync.dma_start(out=idx_res[:], in_=idxr)
    m_res = const_pool.tile([P, SEQ_LEN, P], bf16)
    nc.sync.dma_start(out=m_res[:], in_=mr)
    tf_res = const_pool.tile([P, nb, T], f32)
    nc.sync.dma_start(out=tf_res[:], in_=tfr)
    pad_t = const_pool.tile([P, H + T], f32)
    nc.sync.dma_start(out=pad_t[:], in_=padr)

    ob_tiles = {}

    def finish_B(Bex):
        t, seen_j = ob_tiles.pop(Bex)
        for jj in range(SEQ_LEN):
            if jj not in seen_j:
                nc.scalar.mul(t[:, jj, :], pad_t[:], 1.0)
        e0 = Bex * P
        npar = plan.npar_of_B[Bex]
        nc.sync.dma_start(out=out[e0 : e0 + npar], in_=t[:npar])

    for bi in range(nb):
        Bex, j = plan.block_Bj[bi]
        if Bex not in ob_tiles:
            ob = ob_pool.tile(
                [P, SEQ_LEN, H + T], f32, tag="ob", name=f"ob_{Bex}"
            )
            ob_tiles[Bex] = (ob, set())
        ob, seen_j = ob_tiles[Bex]
        seen_j.add(j)

        nrows = plan.npar_of_B[Bex] * NODES_PER_SEG   # 1280 or 680
        ntiles = (nrows + P - 1) // P
        acc = ps_pool.tile([P, H], f32, tag="ps", name=f"acc_{bi}")
        g = g_pool.tile([P, SEQ_LEN, H], bf16, tag="g", name=f"g_{bi}")
        for t in range(ntiles):
            npr = min(P, nrows - t * P)
            nc.gpsimd.indirect_dma_start(
                out=g[:npr, t, :],
                out_offset=None,
                in_=table,
                in_offset=bass.IndirectOffsetOnAxis(
                    ap=idx_res[:npr, bi * SEQ_LEN + t : bi * SEQ_LEN + t + 1],
                    axis=0,
                ),
            )
        for t in range(ntiles):
            npr = min(P, nrows - t * P)
            nc.tensor.matmul(
                acc[:],
                m_res[:npr, t, :],
                g[:npr, t, :],
                start=(t == 0),
                stop=(t == ntiles - 1),
            )
        nc.scalar.mul(ob[:, j, 0:H], acc[:], 1.0 / NODES_PER_SEG)
        nc.scalar.mul(ob[:, j, H : H + T], tf_res[:, bi, :], 1.0)

        last_of_B = bi == nb - 1 or plan.block_Bj[bi + 1][0] != Bex
        if last_of_B:
            finish_B(Bex)

    # example blocks with no compute blocks at all
    covered = {Bj[0] for Bj in plan.block_Bj}
    for Bex in range(NB_EX):
        if Bex not in covered:
            ob = ob_pool.tile(
                [P, SEQ_LEN, H + T], f32, tag="ob", name=f"obp_{Bex}"
            )
            ob_tiles[Bex] = (ob, set())
            finish_B(Bex)


def _build_nc(plan):
    nc = bacc.Bacc(
        "TRN2",
        target_bir_lowering=False,
        debug=False,
        enable_asserts=False,
        num_devices=N_CORES,
    )
    f32 = mybir.dt.float32
    table = nc.dram_tensor(
        "table", [NUM_ENTITIES + 1, H], mybir.dt.bfloat16, kind="ExternalInput"
    ).ap()
    idxr = nc.dram_tensor(
        "idxr", [P, plan.nb * SEQ_LEN], mybir.dt.int32, kind="ExternalInput"
    ).ap()
    mr = nc.dram_tensor(
        "mr", [P, SEQ_LEN, P], mybir.dt.bfloat16, kind="ExternalInput"
    ).ap()
    tfr = nc.dram_tensor(
        "tfr", [P, plan.nb, T], f32, kind="ExternalInput"
    ).ap()
    padr = nc.dram_tensor("padr", [P, H + T], f32, kind="ExternalInput").ap()
    out = nc.dram_tensor(
        "out", [EX_PER_CORE, SEQ_LEN, H + T], f32, kind="ExternalOutput"
    ).ap()
    with tile.TileContext(nc) as tc:
        _emit(tc, plan, table, idxr, mr, tfr, padr, out)
    nc.compile()
    return nc


def kernel(
    ent_embeds, t_w, t_b, flat_s, node_seg_ids, seg_example, seg_pos, time_vals
):
    ent_embeds = np.ascontiguousarray(ent_embeds, dtype=np.float32)
    t_w = np.asarray(t_w, dtype=np.float32)
    t_b = np.asarray(t_b, dtype=np.float32)
    flat_s = np.asarray(flat_s, dtype=np.int32)
    node_seg_ids = np.asarray(node_seg_ids, dtype=np.int32)
    seg_example = np.asarray(seg_example, dtype=np.int32)
    seg_pos = np.asarray(seg_pos, dtype=np.int32)
    time_vals = np.asarray(time_vals, dtype=np.int32)

    plan, idx_hosts, m_host, tf_hosts, pad_host = _host_prep(
        t_w, t_b, flat_s, node_seg_ids, seg_example, seg_pos, time_vals
    )
    table_bf16 = np.zeros((NUM_ENTITIES + 1, H), ml_dtypes.bfloat16)
    table_bf16[:NUM_ENTITIES] = ent_embeds.astype(ml_dtypes.bfloat16)

    if "nc" not in _CACHE:
        _CACHE["nc"] = _build_nc(plan)
    nc = _CACHE["nc"]

    in_maps = []
    for c in range(N_CORES):
        in_maps.append(
            {
                "table": table_bf16,
                "idxr": idx_hosts[c],
                "mr": m_host,
                "tfr": tf_hosts[c],
                "padr": pad_host,
            }
        )

    trace = os.environ.get("BASSKERNEL_TRACE", "0") == "1"
    kw = {}
    if trace:
        kw = dict(trace=True, tmpdir=os.environ.get("BASSKERNEL_TRACEDIR") or None)
    res = run_bass_kernel_spmd(nc, in_maps, core_ids=list(range(N_CORES)), **kw)
    if trace:
        _CACHE["last_results"] = res
        print(f"[kernel] exec_time_ns={res.exec_time_ns}", file=sys.stderr)

    shards = [res.results[c]["out"] for c in range(N_CORES)]
    return np.concatenate(shards, axis=0)


# revision 11
# speedup vs baseline: 1.1322x; 1.1322x over previous
"""Trainium2 Bass kernel for nn_MeanAggregator (segment mean + time features).

Computation (see reference):
  out[e, p, 0:256]   = mean of 10 gathered ent_embeds rows of segment 5e+p (p<5)
  out[e, p, 256:288] = cos(t * t_w + t_b), t = time_vals[5e+p]             (p<5)
  out[e, p, 0:256]   = 0,  out[e, p, 256:288] = cos(1e6*t_w + t_b)         (p>=5)

Sharding: data-parallel by segment range; core c owns segments
[12500c, 12500(c+1)) == examples [2500c, 2500(c+1)).

Hardware reality (measured): data-dependent DMA is emission-bound on the
Q7 SWDGE.  [128,1]-offset indirect DMA costs ~1.40 us per 128 rows
(10.95 ns/row incl. dispatch); InstDMAGatherAnt costs ~683 ns/call +
7.94 ns/index.  Big dma_gather calls therefore win IF index-stream padding
is kept small.  Design:
  - Table staged bf16; split into 7 chunks of <=32768 rows so dma_gather's
    int16 indices can address them; idx 0 used for padding (dead slot id).
  - Blocks of 128 segments: block (B, j) = seq-pos j of examples
    [128B, 128B+128), slot = example offset.  Node stream is grouped
    (8 blocks)-major, then chunk, then block; per-(block, chunk) piece
    sizes are padded only to the max across the 8 cores (~+10%) so one
    SPMD program fits every core.
  - One dma_gather per (group, chunk) (~1.8k indices).  Rows land at
    call-stream position r -> [r%128, r//128] of the call tile; pieces are
    NOT 128-aligned, so a 128-row tile column may hold rows of several
    blocks.  Routing: per (column, block) pair a one-hot M is built on DVE
    with one dual-op tensor_scalar, M[p, s] = (iota[s] + 128*g_b ==
    gslot[p]) (group-local slot ids; pad rows get 9999 -> all-zero M row),
    and TensorE accumulates M^T @ column into the block's PSUM tile.
  - ScalarE evicts PSUM * 0.1 into a [128, 10, 288] per-example-block
    tile (+ host-LUT time features, pad halves); one 1.44 MB DMA per
    example block writes the output.
"""

import math
import os
import sys

import numpy as np

sys.path.insert(0, "/opt/trn_rl_repo")

from contextlib import ExitStack

import ml_dtypes

import concourse.bass as bass
import concourse.tile as tile
from concourse import bacc, mybir
from concourse._compat import with_exitstack
from concourse.bass_utils import run_bass_kernel_spmd
from concourse.library_config import mlp

# Problem constants (hardcoded; kernel.py must be self-contained).
N_CORES = 8
NUM_ENTITIES = 200000
H = 256
T = 32
SEQ_LEN = 10
N_EXAMPLES = 20000
SEGS_PER_EX = 5
NODES_PER_SEG = 10
N_SEG = N_EXAMPLES * SEGS_PER_EX
EX_PER_CORE = N_EXAMPLES // N_CORES      # 2500
NSEG_CORE = N_SEG // N_CORES             # 12500
P = 128
NB_EX = (EX_PER_CORE + P - 1) // P       # 20 example blocks per core
CHUNK = 32768
NCHUNK = (NUM_ENTITIES + CHUNK - 1) // CHUNK  # 7
GROUP = 8
DEAD_SLOT = 9999.0
PAD_TIME = 1000000.0

_CACHE = {}


class _Plan:
    pass


def _host_prep(t_w, t_b, flat_s, node_seg_ids, seg_example, seg_pos, time_vals):
    """Shared static plan + per-core device input arrays."""
    e = seg_example.astype(np.int64)
    j = seg_pos.astype(np.int64)
    segs = np.arange(N_SEG, dtype=np.int64)
    core_of_seg = segs // NSEG_CORE
    e_loc = e - core_of_seg * EX_PER_CORE
    assert e_loc.min() >= 0 and e_loc.max() < EX_PER_CORE
    B = e_loc // P
    slot = e_loc % P
    segkey = B * SEQ_LEN + j
    uniq = ((core_of_seg * NB_EX + B) * SEQ_LEN + j) * P + slot
    assert np.unique(uniq).size == N_SEG, "duplicate (example, pos) targets"

    block_keys = np.unique(segkey)
    nb = int(block_keys.size)
    bi_of_key = np.full(NB_EX * SEQ_LEN, -1, np.int64)
    bi_of_key[block_keys] = np.arange(nb)
    seg_bi = bi_of_key[segkey]

    # nodes -> (core, block, chunk, slot, local idx)
    nseg = node_seg_ids.astype(np.int64)
    node_core = nseg // NSEG_CORE
    node_bi = bi_of_key[segkey[nseg]]
    node_slot = slot[nseg]
    fs = flat_s.astype(np.int64)
    node_k = fs // CHUNK
    node_local = (fs - node_k * CHUNK).astype(np.int16)

    pid = (node_core * nb + node_bi) * NCHUNK + node_k
    counts = np.bincount(pid, minlength=N_CORES * nb * NCHUNK).reshape(
        N_CORES, nb, NCHUNK
    )
    Ptab = counts.max(axis=0).astype(np.int64)        # [nb, NCHUNK], no ceil

    # stream order: group-major, then chunk, then block-within-group
    groups = [list(range(g0, min(g0 + GROUP, nb))) for g0 in range(0, nb, GROUP)]
    piece_base = np.zeros((nb, NCHUNK), np.int64)      # call-local row offset
    call_rows = {}                                      # (g,k) -> S_gk
    call_off16 = {}                                     # idx-res col16 offset
    call_off128 = {}                                    # gseg-res col offset
    o16 = 0
    o128 = 0
    for gi, gbl in enumerate(groups):
        for k in range(NCHUNK):
            off = 0
            for b in gbl:
                piece_base[b, k] = off
                off += int(Ptab[b, k])
            call_rows[(gi, k)] = off
            call_off16[(gi, k)] = o16
            call_off128[(gi, k)] = o128
            o16 += (off + 15) // 16
            o128 += (off + P - 1) // P
    tot16 = o16
    tot128 = o128

    # per-node stream position within its call
    sort_idx = np.argsort(pid, kind="stable")
    spid = pid[sort_idx]
    firsts = np.r_[0, np.flatnonzero(np.diff(spid)) + 1]
    runlen = np.diff(np.r_[firsts, spid.size])
    rank = np.arange(spid.size) - np.repeat(firsts, runlen)
    s_bi = (spid // NCHUNK) % nb
    s_k = spid % NCHUNK
    s_core = spid // (nb * NCHUNK)
    s_gi = s_bi // GROUP
    pos = piece_base[s_bi, s_k] + rank                 # call-local row

    # device idx / gslot arrays
    idx_hosts = [np.zeros((P, tot16), np.int16) for _ in range(N_CORES)]
    gseg_hosts = [
        np.full((P, tot128), DEAD_SLOT, np.float32) for _ in range(N_CORES)
    ]
    co16 = np.array(
        [[call_off16[(gi, k)] for k in range(NCHUNK)] for gi in range(len(groups))]
    )
    co128 = np.array(
        [[call_off128[(gi, k)] for k in range(NCHUNK)] for gi in range(len(groups))]
    )
    n_p16 = co16[s_gi, s_k] * 16 + pos                 # global 16-grid position
    n_p128 = co128[s_gi, s_k] * 128 + pos              # global 128-grid position
    sloc = node_local[sort_idx]
    sgslot = ((s_bi % GROUP) * P + node_slot[sort_idx]).astype(np.float32)
    for c in range(N_CORES):
        m = s_core == c
        idx_hosts[c][n_p16[m] % 16, n_p16[m] // 16] = sloc[m]
        gseg_hosts[c][n_p128[m] % 128, n_p128[m] // 128] = sgslot[m]
        # dma_gather reads idxs from each Q7 core's own 16-partition group:
        # replicate rows 0..15 across all 8 groups.
        idx_hosts[c] = np.ascontiguousarray(np.tile(idx_hosts[c][:16], (8, 1)))
        gseg_hosts[c] = np.ascontiguousarray(gseg_hosts[c])
    # NOTE: idx wrap is PER CALL: position i of call -> [i%16, base+i//16].
    # n_p16 = base*16 + i, so [n_p16%16, n_p16//16] is correct only if
    # i%16 == n_p16%16, i.e. base*16 % 16 == 0 -- true by construction.

    # time features / pad / iota
    t_w32 = t_w.astype(np.float32)
    t_b32 = t_b.astype(np.float32)
    pad_vec = np.cos(np.float32(PAD_TIME) * t_w32 + t_b32).astype(np.float32)
    tf_seg = np.cos(
        time_vals.astype(np.float32)[:, None] * t_w32 + t_b32
    ).astype(np.float32)
    tf_all = np.tile(pad_vec, (N_CORES, P, nb, 1)).astype(np.float32)
    tf_all[core_of_seg, slot, seg_bi] = tf_seg
    tf_hosts = [np.ascontiguousarray(tf_all[c]) for c in range(N_CORES)]
    pad_host = np.zeros((P, H + T), np.float32)
    pad_host[:, H:] = pad_vec
    iota_host = np.tile(
        np.arange(P, dtype=np.float32), (P, 1)
    ).astype(ml_dtypes.bfloat16)

    plan = _Plan()
    plan.nb = nb
    plan.groups = groups
    plan.Ptab = Ptab
    plan.piece_base = piece_base
    plan.call_rows = call_rows
    plan.call_off16 = call_off16
    plan.call_off128 = call_off128
    plan.tot16 = tot16
    plan.tot128 = tot128
    plan.block_Bj = [(int(kk) // SEQ_LEN, int(kk) % SEQ_LEN) for kk in block_keys]
    plan.npar_of_B = [min(P, EX_PER_CORE - Bx * P) for Bx in range(NB_EX)]
    return plan, idx_hosts, gseg_hosts, tf_hosts, pad_host, iota_host


@with_exitstack
def _emit(ctx: ExitStack, tc, plan, table, idxr, gsegr, iotar, tfr, padr, out):
    nc = tc.nc
    f32 = mybir.dt.float32
    bf16 = mybir.dt.bfloat16
    nb = plan.nb

    const_pool = ctx.enter_context(tc.tile_pool(name="const", bufs=1))
    gd_pool = ctx.enter_context(tc.tile_pool(name="gd", bufs=2))
    m_pool = ctx.enter_context(tc.tile_pool(name="m", bufs=8))
    ob_pool = ctx.enter_context(tc.tile_pool(name="ob", bufs=3))
    ps_pool = ctx.enter_context(tc.tile_pool(name="ps", bufs=8, space="PSUM"))

    nc.gpsimd.load_library(mlp)

    idx_res = const_pool.tile([P, plan.tot16], mybir.dt.int16)
    nc.sync.dma_start(out=idx_res[:], in_=idxr)
    gseg_res = const_pool.tile([P, plan.tot128], f32)
    nc.sync.dma_start(out=gseg_res[:], in_=gsegr)
    iota_t = const_pool.tile([P, P], bf16)
    nc.sync.dma_start(out=iota_t[:], in_=iotar)
    tf_res = const_pool.tile([P, nb, T], f32)
    nc.sync.dma_start(out=tf_res[:], in_=tfr)
    pad_t = const_pool.tile([P, H + T], f32)
    nc.sync.dma_start(out=pad_t[:], in_=padr)

    # max call tile columns per chunk tag (for pool sizing)
    maxcols = {
        k: max(
            (plan.call_rows[(gi, k)] + P - 1) // P
            for gi in range(len(plan.groups))
        )
        for k in range(NCHUNK)
    }

    ob_tiles = {}

    def finish_B(Bex):
        t, seen_j = ob_tiles.pop(Bex)
        for jj in range(SEQ_LEN):
            if jj not in seen_j:
                nc.scalar.mul(t[:, jj, :], pad_t[:], 1.0)
        e0 = Bex * P
        npar = plan.npar_of_B[Bex]
        nc.sync.dma_start(out=out[e0 : e0 + npar], in_=t[:npar])

    for gi, gbl in enumerate(plan.groups):
        # gather calls, one per chunk
        gts = {}
        for k in range(NCHUNK):
            S = plan.call_rows[(gi, k)]
            if S == 0:
                continue
            gt = gd_pool.tile(
                [P, maxcols[k], H], bf16, tag=f"gd{k}", name=f"gd_{gi}_{k}"
            )
            c0 = CHUNK * k
            crows = min(CHUNK, NUM_ENTITIES - c0)
            o16 = plan.call_off16[(gi, k)]
            nc.gpsimd.dma_gather(
                gt[:, : (S + P - 1) // P, :],
                table[c0 : c0 + crows, :],
                idx_res[:, o16 : o16 + (S + 15) // 16],
                S,
                S,
                H,
                single_packet=False,
            )
            gts[k] = gt

        # compute per block: walk its pieces' tile columns
        for b in gbl:
            Bex, j = plan.block_Bj[b]
            if Bex not in ob_tiles:
                ob = ob_pool.tile(
                    [P, SEQ_LEN, H + T], f32, tag="ob", name=f"ob_{Bex}"
                )
                ob_tiles[Bex] = (ob, set())
            ob, seen_j = ob_tiles[Bex]
            seen_j.add(j)
            g_b = b % GROUP

            # static (chunk, col) pairs this block contributes to
            pairs = []
            for k in range(NCHUNK):
                pl = int(plan.Ptab[b, k])
                if pl == 0:
                    continue
                r0 = int(plan.piece_base[b, k])
                r1 = r0 + pl
                for col in range(r0 // P, (r1 + P - 1) // P):
                    pairs.append((k, col))

            if not pairs:
                nc.scalar.mul(ob[:, j, 0:H], pad_t[:, 0:H], 1.0)
            else:
                acc = ps_pool.tile([P, H], f32, tag="ps", name=f"acc_{b}")
                for i, (k, col) in enumerate(pairs):
                    o128 = plan.call_off128[(gi, k)]
                    m = m_pool.tile([P, P], bf16, tag="m", name=f"m_{b}_{i}")
                    nc.vector.tensor_scalar(
                        out=m[:],
                        in0=iota_t[:],
                        scalar1=float(g_b * P),
                        scalar2=gseg_res[:, o128 + col : o128 + col + 1],
                        op0=mybir.AluOpType.add,
                        op1=mybir.AluOpType.is_equal,
                    )
                    nc.tensor.matmul(
                        acc[:],
                        m[:],
                        gts[k][:, col, :],
                        start=(i == 0),
                        stop=(i == len(pairs) - 1),
                    )
                nc.scalar.mul(ob[:, j, 0:H], acc[:], 1.0 / NODES_PER_SEG)
            nc.scalar.mul(ob[:, j, H : H + T], tf_res[:, b, :], 1.0)

            last_of_B = b == nb - 1 or plan.block_Bj[b + 1][0] != Bex
            if last_of_B:
                finish_B(Bex)

    covered = {Bj[0] for Bj in plan.block_Bj}
    for Bex in range(NB_EX):
        if Bex not in covered:
            ob = ob_pool.tile(
                [P, SEQ_LEN, H + T], f32, tag="ob", name=f"obp_{Bex}"
            )
            ob_tiles[Bex] = (ob, set())
            finish_B(Bex)


def _build_nc(plan):
    nc = bacc.Bacc(
        "TRN2",
        target_bir_lowering=False,
        debug=False,
        enable_asserts=False,
        num_devices=N_CORES,
    )
    f32 = mybir.dt.float32
    bf16 = mybir.dt.bfloat16
    table = nc.dram_tensor(
        "table", [NUM_ENTITIES, H], bf16, kind="ExternalInput"
    ).ap()
    idxr = nc.dram_tensor(
        "idxr", [P, plan.tot16], mybir.dt.int16, kind="ExternalInput"
    ).ap()
    gsegr = nc.dram_tensor(
        "gsegr", [P, plan.tot128], f32, kind="ExternalInput"
    ).ap()
    iotar = nc.dram_tensor("iotar", [P, P], bf16, kind="ExternalInput").ap()
    tfr = nc.dram_tensor("tfr", [P, plan.nb, T], f32, kind="ExternalInput").ap()
    padr = nc.dram_tensor("padr", [P, H + T], f32, kind="ExternalInput").ap()
    out = nc.dram_tensor(
        "out", [EX_PER_CORE, SEQ_LEN, H + T], f32, kind="ExternalOutput"
    ).ap()
    with tile.TileContext(nc) as tc:
        _emit(tc, plan, table, idxr, gsegr, iotar, tfr, padr, out)
    nc.compile()
    return nc


def kernel(
    ent_embeds, t_w, t_b, flat_s, node_seg_ids, seg_example, seg_pos, time_vals
):
    ent_embeds = np.ascontiguousarray(ent_embeds, dtype=np.float32)
    t_w = np.asarray(t_w, dtype=np.float32)
    t_b = np.asarray(t_b, dtype=np.float32)
    flat_s = np.asarray(flat_s, dtype=np.int32)
    node_seg_ids = np.asarray(node_seg_ids, dtype=np.int32)
    seg_example = np.asarray(seg_example, dtype=np.int32)
    seg_pos = np.asarray(seg_pos, dtype=np.int32)
    time_vals = np.asarray(time_vals, dtype=np.int32)

    plan, idx_hosts, gseg_hosts, tf_hosts, pad_host, iota_host = _host_prep(
        t_w, t_b, flat_s, node_seg_ids, seg_example, seg_pos, time_vals
    )
    table_bf16 = ent_embeds.astype(ml_dtypes.bfloat16)

    if "nc" not in _CACHE:
        _CACHE["nc"] = _build_nc(plan)
    nc = _CACHE["nc"]

    in_maps = []
    for c in range(N_CORES):
        in_maps.append(
            {
                "table": table_bf16,
                "idxr": idx_hosts[c],
                "gsegr": gseg_hosts[c],
                "iotar": iota_host,
                "tfr": tf_hosts[c],
                "padr": pad_host,
            }
        )

    trace = os.environ.get("BASSKERNEL_TRACE", "0") == "1"
    kw = {}
    if trace:
        kw = dict(trace=True, tmpdir=os.environ.get("BASSKERNEL_TRACEDIR") or None)
    res = run_bass_kernel_spmd(nc, in_maps, core_ids=list(range(N_CORES)), **kw)
    if trace:
        _CACHE["last_results"] = res
        print(f"[kernel] exec_time_ns={res.exec_time_ns}", file=sys.stderr)

    shards = [res.results[c]["out"] for c in range(N_CORES)]
    return np.concatenate(shards, axis=0)


# revision 12
# speedup vs baseline: 1.1788x; 1.0412x over previous
"""Trainium2 Bass kernel for nn_MeanAggregator (segment mean + time features).

Computation (see reference):
  out[e, p, 0:256]   = mean of 10 gathered ent_embeds rows of segment 5e+p (p<5)
  out[e, p, 256:288] = cos(t * t_w + t_b), t = time_vals[5e+p]             (p<5)
  out[e, p, 0:256]   = 0,  out[e, p, 256:288] = cos(1e6*t_w + t_b)         (p>=5)

Sharding: data-parallel by segment range; core c owns segments
[12500c, 12500(c+1)) == examples [2500c, 2500(c+1)).

Hardware reality (measured): every data-dependent DMA path (indirect DMA,
dma_gather) is emission-bound on the Q7 SWDGE at ~8.6 ns/descriptor, one
row per descriptor, engine-serial.  125k gathered rows/core -> ~1.07 ms of
GpSimd time is the floor; the job of the rest of the kernel is to stay out
of the way.  Design:
  - Table staged as bf16 (+1 trailing zero row for absent segments).
  - Host reorders the node stream block-major: block (B, j) = seq-pos j of
    examples [128B, 128B+128); its 1280 node rows are slot-major, so row r
    of the block belongs to output slot r//10.  That mapping is STATIC:
    ten precomputed one-hot matrices M_t[p, s] = ((128t+p)//10 == s) turn
    the per-block segment sum into 10 PSUM-accumulated TensorE matmuls.
    No per-tile index compute on DVE at all.
  - Gathers are [128,1]-offset indirect DMAs (the only HW-honored form),
    one per block tile: 10 per block, partial tail tiles for the last
    example block, exactly 125000 descriptors per core.
  - ScalarE evicts PSUM * 0.1 into a [128, 10, 288] per-example-block tile
    (plus host-LUT time features and pad halves); one 1.44 MB DMA per
    example block writes the output.
"""

import math
import os
import sys

import numpy as np

sys.path.insert(0, "/opt/trn_rl_repo")

from contextlib import ExitStack

import ml_dtypes

import concourse.bass as bass
import concourse.tile as tile
from concourse import bacc, mybir
from concourse._compat import with_exitstack
from concourse.bass_utils import run_bass_kernel_spmd

# Problem constants (hardcoded; kernel.py must be self-contained).
N_CORES = 8
NUM_ENTITIES = 200000
H = 256
T = 32
SEQ_LEN = 10
N_EXAMPLES = 20000
SEGS_PER_EX = 5
NODES_PER_SEG = 10
N_SEG = N_EXAMPLES * SEGS_PER_EX
N_NODES = N_SEG * NODES_PER_SEG
EX_PER_CORE = N_EXAMPLES // N_CORES      # 2500
NSEG_CORE = N_SEG // N_CORES             # 12500
P = 128
NB_EX = (EX_PER_CORE + P - 1) // P       # 20 example blocks per core
ZROW = NUM_ENTITIES                       # index of the staged all-zero row
PAD_TIME = 1000000.0

_CACHE = {}


class _Plan:
    def __init__(self, block_keys, block_Bj, nb, npar_of_B):
        self.block_keys = block_keys
        self.block_Bj = block_Bj
        self.nb = nb
        self.npar_of_B = npar_of_B


def _host_prep(t_w, t_b, flat_s, node_seg_ids, seg_example, seg_pos, time_vals):
    """Shared static plan + per-core device input arrays."""
    e = seg_example.astype(np.int64)
    j = seg_pos.astype(np.int64)
    segs = np.arange(N_SEG, dtype=np.int64)
    core_of_seg = segs // NSEG_CORE
    e_loc = e - core_of_seg * EX_PER_CORE
    assert e_loc.min() >= 0 and e_loc.max() < EX_PER_CORE, (
        "segment's example outside its core's range; resharding needed"
    )
    B = e_loc // P
    slot = e_loc % P
    segkey = B * SEQ_LEN + j
    uniq = ((core_of_seg * NB_EX + B) * SEQ_LEN + j) * P + slot
    assert np.unique(uniq).size == N_SEG, "duplicate (example, pos) targets"

    block_keys = np.unique(segkey)
    nb = int(block_keys.size)
    bi_of_key = np.full(NB_EX * SEQ_LEN, -1, np.int64)
    bi_of_key[block_keys] = np.arange(nb)
    seg_bi = bi_of_key[segkey]

    # node rows per segment (node_seg_ids is sorted)
    nseg = node_seg_ids.astype(np.int64)
    starts = np.searchsorted(nseg, segs)
    ends = np.searchsorted(nseg, segs, side="right")
    assert ((ends - starts) == NODES_PER_SEG).all(), (
        "kernel assumes exactly 10 nodes per segment"
    )

    # per-core block-major node stream: idx_blk[core][bi, slot, node]
    fs = flat_s.astype(np.int32)
    idx_blk = np.full((N_CORES, nb, P, NODES_PER_SEG), ZROW, np.int32)
    node_rows = starts[:, None] + np.arange(NODES_PER_SEG)[None, :]  # [N_SEG,10]
    idx_blk[core_of_seg, seg_bi, slot] = fs[node_rows]

    # gather-call layout: idx_res[p, bi*10 + t] = stream row 128t+p of block
    # (stream = slot-major flatten of [P, 10])
    idx_hosts = []
    for c in range(N_CORES):
        stream = idx_blk[c].reshape(nb, P * NODES_PER_SEG)       # [nb, 1280]
        tiles = stream.reshape(nb, SEQ_LEN, P)                    # [nb, 10t, 128p]
        idx_hosts.append(
            np.ascontiguousarray(
                np.transpose(tiles, (2, 0, 1)).reshape(P, nb * SEQ_LEN)
            )
        )

    # static one-hot M_t[p, s] = ((128t+p)//10 == s)
    m_host = np.zeros((P, SEQ_LEN, P), np.float32)
    for t in range(SEQ_LEN):
        r = 128 * t + np.arange(P)
        s = r // NODES_PER_SEG
        ok = s < P
        m_host[np.arange(P)[ok], t, s[ok]] = 1.0
    m_host = m_host.astype(ml_dtypes.bfloat16)

    # time features per (core, block): [P, nb, T] f32, pad_vec default
    t_w32 = t_w.astype(np.float32)
    t_b32 = t_b.astype(np.float32)
    pad_vec = np.cos(np.float32(PAD_TIME) * t_w32 + t_b32).astype(np.float32)
    tf_seg = np.cos(
        time_vals.astype(np.float32)[:, None] * t_w32 + t_b32
    ).astype(np.float32)
    tf_all = np.tile(pad_vec, (N_CORES, P, nb, 1)).astype(np.float32)
    tf_all[core_of_seg, slot, seg_bi] = tf_seg
    tf_hosts = [np.ascontiguousarray(tf_all[c]) for c in range(N_CORES)]

    pad_host = np.zeros((P, H + T), np.float32)
    pad_host[:, H:] = pad_vec

    block_Bj = [(int(k) // SEQ_LEN, int(k) % SEQ_LEN) for k in block_keys]
    npar_of_B = [min(P, EX_PER_CORE - Bx * P) for Bx in range(NB_EX)]
    plan = _Plan(block_keys, block_Bj, nb, npar_of_B)
    return plan, idx_hosts, m_host, tf_hosts, pad_host


@with_exitstack
def _emit(ctx: ExitStack, tc, plan, table, idxr, mr, tfr, padr, out):
    nc = tc.nc
    f32 = mybir.dt.float32
    bf16 = mybir.dt.bfloat16
    nb = plan.nb

    const_pool = ctx.enter_context(tc.tile_pool(name="const", bufs=1))
    g_pool = ctx.enter_context(tc.tile_pool(name="g", bufs=48))
    ob_pool = ctx.enter_context(tc.tile_pool(name="ob", bufs=3))
    ps_pool = ctx.enter_context(tc.tile_pool(name="ps", bufs=4, space="PSUM"))

    idx_res = const_pool.tile([P, nb * SEQ_LEN], mybir.dt.int32)
    nc.sync.dma_start(out=idx_res[:], in_=idxr)
    m_res = const_pool.tile([P, SEQ_LEN, P], bf16)
    nc.sync.dma_start(out=m_res[:], in_=mr)
    tf_res = const_pool.tile([P, nb, T], f32)
    nc.sync.dma_start(out=tf_res[:], in_=tfr)
    pad_t = const_pool.tile([P, H + T], f32)
    nc.sync.dma_start(out=pad_t[:], in_=padr)

    ob_tiles = {}

    def finish_B(Bex):
        t, seen_j = ob_tiles.pop(Bex)
        for jj in range(SEQ_LEN):
            if jj not in seen_j:
                nc.scalar.mul(t[:, jj, :], pad_t[:], 1.0)
        e0 = Bex * P
        npar = plan.npar_of_B[Bex]
        nc.sync.dma_start(out=out[e0 : e0 + npar], in_=t[:npar])

    for bi in range(nb):
        Bex, j = plan.block_Bj[bi]
        if Bex not in ob_tiles:
            ob = ob_pool.tile(
                [P, SEQ_LEN, H + T], f32, tag="ob", name=f"ob_{Bex}"
            )
            ob_tiles[Bex] = (ob, set())
        ob, seen_j = ob_tiles[Bex]
        seen_j.add(j)

        nrows = plan.npar_of_B[Bex] * NODES_PER_SEG   # 1280 or 680
        ntiles = (nrows + P - 1) // P
        acc = ps_pool.tile([P, H], f32, tag="ps", name=f"acc_{bi}")
        for t in range(ntiles):
            npr = min(P, nrows - t * P)
            g = g_pool.tile([P, H], bf16, tag="g", name=f"g_{bi}_{t}")
            nc.gpsimd.indirect_dma_start(
                out=g[:npr],
                out_offset=None,
                in_=table,
                in_offset=bass.IndirectOffsetOnAxis(
                    ap=idx_res[:npr, bi * SEQ_LEN + t : bi * SEQ_LEN + t + 1],
                    axis=0,
                ),
            )
            nc.tensor.matmul(
                acc[:],
                m_res[:npr, t, :],
                g[:npr],
                start=(t == 0),
                stop=(t == ntiles - 1),
            )
        nc.scalar.mul(ob[:, j, 0:H], acc[:], 1.0 / NODES_PER_SEG)
        nc.scalar.mul(ob[:, j, H : H + T], tf_res[:, bi, :], 1.0)

        last_of_B = bi == nb - 1 or plan.block_Bj[bi + 1][0] != Bex
        if last_of_B:
            finish_B(Bex)

    # example blocks with no compute blocks at all
    covered = {Bj[0] for Bj in plan.block_Bj}
    for Bex in range(NB_EX):
        if Bex not in covered:
            ob = ob_pool.tile(
                [P, SEQ_LEN, H + T], f32, tag="ob", name=f"obp_{Bex}"
            )
            ob_tiles[Bex] = (ob, set())
            finish_B(Bex)


def _build_nc(plan):
    nc = bacc.Bacc(
        "TRN2",
        target_bir_lowering=False,
        debug=False,
        enable_asserts=False,
        num_devices=N_CORES,
    )
    f32 = mybir.dt.float32
    table = nc.dram_tensor(
        "table", [NUM_ENTITIES + 1, H], mybir.dt.bfloat16, kind="ExternalInput"
    ).ap()
    idxr = nc.dram_tensor(
        "idxr", [P, plan.nb * SEQ_LEN], mybir.dt.int32, kind="ExternalInput"
    ).ap()
    mr = nc.dram_tensor(
        "mr", [P, SEQ_LEN, P], mybir.dt.bfloat16, kind="ExternalInput"
    ).ap()
    tfr = nc.dram_tensor(
        "tfr", [P, plan.nb, T], f32, kind="ExternalInput"
    ).ap()
    padr = nc.dram_tensor("padr", [P, H + T], f32, kind="ExternalInput").ap()
    out = nc.dram_tensor(
        "out", [EX_PER_CORE, SEQ_LEN, H + T], f32, kind="ExternalOutput"
    ).ap()
    with tile.TileContext(nc) as tc:
        _emit(tc, plan, table, idxr, mr, tfr, padr, out)
    nc.compile()
    return nc


def kernel(
    ent_embeds, t_w, t_b, flat_s, node_seg_ids, seg_example, seg_pos, time_vals
):
    ent_embeds = np.ascontiguousarray(ent_embeds, dtype=np.float32)
    t_w = np.asarray(t_w, dtype=np.float32)
    t_b = np.asarray(t_b, dtype=np.float32)
    flat_s = np.asarray(flat_s, dtype=np.int32)
    node_seg_ids = np.asarray(node_seg_ids, dtype=np.int32)
    seg_example = np.asarray(seg_example, dtype=np.int32)
    seg_pos = np.asarray(seg_pos, dtype=np.int32)
    time_vals = np.asarray(time_vals, dtype=np.int32)

    plan, idx_hosts, m_host, tf_hosts, pad_host = _host_prep(
        t_w, t_b, flat_s, node_seg_ids, seg_example, seg_pos, time_vals
    )
    table_bf16 = np.zeros((NUM_ENTITIES + 1, H), ml_dtypes.bfloat16)
    table_bf16[:NUM_ENTITIES] = ent_embeds.astype(ml_dtypes.bfloat16)

    if "nc" not in _CACHE:
        _CACHE["nc"] = _build_nc(plan)
    nc = _CACHE["nc"]

    in_maps = []
    for c in range(N_CORES):
        in_maps.append(
            {
                "table": table_bf16,
                "idxr": idx_hosts[c],
                "mr": m_host,
                "tfr": tf_hosts[c],
                "padr": pad_host,
            }
        )

    trace = os.environ.get("BASSKERNEL_TRACE", "0") == "1"
    kw = {}
    if trace:
        kw = dict(trace=True, tmpdir=os.environ.get("BASSKERNEL_TRACEDIR") or None)
    res = run_bass_kernel_spmd(nc, in_maps, core_ids=list(range(N_CORES)), **kw)
    if trace:
        _CACHE["last_results"] = res
        print(f"[kernel] exec_time_ns={res.exec_time_ns}", file=sys.stderr)

    shards = [res.results[c]["out"] for c in range(N_CORES)]
    return np.concatenate(shards, axis=0)
